# revision 24
# baseline (speedup 1.0000x reference)
"""Multi-head attention (B=4, S=2048, D=1024, H=16) on 8 NeuronCores.

Reference quirk: the key-padding mask uses jnp.tile(valid_length, H) indexed
by the flat (b*H + h) head-batch index, so the effective mask length for
(batch b, head h) is valid_length[h % 4] -- it depends on the head CLASS
(h mod 4), not the batch.

Sharding: core i handles batch i%4 and the 8 heads {4P..4P+3, 4P+8..4P+11}
(P = i//4).  Those 8 heads contain each mask class exactly twice, so every
core does identical work (load-balanced by construction), and key/value work
beyond valid_length[class] (rounded up to 128) is skipped entirely.  The two
same-class heads (h, h+8) are row-packed into one 64-contraction PE pair.
Per-core partial outputs (rank-512 contributions through Wo) are summed on
the host (cores i and i+4 hold the two halves of batch i%4's heads).

All matmuls run in bf16 (fp32 PSUM accumulation).  Attention is computed in
"transposed" orientation S^T[k, q] so that softmax masking is a per-partition
exp bias, the k-sum comes free via an appended ones-column on V, and no
on-chip transposes are needed anywhere.

The emission is software-pipelined around the exp bottleneck: the ACT engine
(the only engine that can do exp) needs ~1.15us per 128-key tile while the
PE produces that tile's scores in ~0.43us, so between score tiles the PE is
fed "filler" work -- PV accumulation of the previous (qb, slot) step, the
merged V projection, late Q-projection chunks, and Wo output tiles -- from a
FIFO whose order also guarantees the in-order PE queue can never wait on an
instruction queued behind it.  Wo results stream straight from PSUM to DRAM
in fp32 (no staging copy).
"""

import sys

for _p in ("/opt/trn_rl_repo", "/root/.axon_site/_ro/trn_rl_repo"):
    if _p not in sys.path:
        sys.path.insert(0, _p)

import numpy as np
import ml_dtypes

B, S, D, H = 4, 2048, 1024, 16
HD = D // H  # 64
NCORES = 8
NSLOT = 4  # head classes (h % 4) per core, 2 heads each
KT = 128  # k-tile size
QB = 512  # q block
KC = 512  # k/q DMA chunk width
MASK_BIAS = -30000.0  # exp(s/8 + bias) == 0 for masked rows (s/8 is O(10))

_compiled = {}  # (T0,T1,T2,T3) -> compiled nc


def core_heads(core):
    """The 8 heads of `core`, in (slot, pair) order: [hA0, hB0, hA1, ...]."""
    P = core // 4
    heads = []
    for c in range(NSLOT):
        heads += [c + 4 * P, c + 8 + 4 * P]
    return heads


def vsort_order(Ts):
    """Slots sorted by descending k-tile count (ties by slot index)."""
    return sorted(range(NSLOT), key=lambda s: (-Ts[s], s))


def _build(Ts, bench_iters=0):
    """Build + compile the single SPMD program for k-tile class profile Ts.

    bench_iters > 0 wraps the whole body in a hardware loop for timing.
    """
    import contextlib
    import concourse.bacc as bacc
    import concourse.tile as tile
    import concourse.mybir as mybir

    fp32 = mybir.dt.float32
    bf16 = mybir.dt.bfloat16

    CKMAX = max(Ts) * KT
    DT = D // 128  # 8 contraction tiles for the projections
    HPC2 = 2 * NSLOT * HD  # 512 head-dim columns per core

    nc = bacc.Bacc("TRN2", target_bir_lowering=False, debug=False, num_devices=NCORES)

    qT = nc.dram_tensor("qT", [D, S], bf16, kind="ExternalInput")
    kT = nc.dram_tensor("kT", [D, CKMAX], bf16, kind="ExternalInput")
    vT = nc.dram_tensor("vT", [D, CKMAX], bf16, kind="ExternalInput")
    wq = nc.dram_tensor("wq", [D, HPC2], bf16, kind="ExternalInput")
    wk = nc.dram_tensor("wk", [D, HPC2], bf16, kind="ExternalInput")
    wv = nc.dram_tensor("wv", [D, HPC2], bf16, kind="ExternalInput")
    wo = nc.dram_tensor("wo", [HPC2, D], bf16, kind="ExternalInput")
    bias_in = nc.dram_tensor("bias", [KT, NSLOT], fp32, kind="ExternalInput")
    fp16 = mybir.dt.float16
    out2 = nc.dram_tensor("out2", [S, D], fp16, kind="ExternalOutput")

    with tile.TileContext(nc) as tc:
        with (
            tc.tile_pool(name="w", bufs=1) as wpool,
            tc.tile_pool(name="xkv", bufs=2) as xkvpool,
            tc.tile_pool(name="xq", bufs=2) as xqpool,
            tc.tile_pool(name="p", bufs=1) as ppool,
            tc.tile_pool(name="qk", bufs=1) as qkpool,
            tc.tile_pool(name="sm", bufs=2) as smpool,
            tc.tile_pool(name="sm1", bufs=1) as sm1pool,
            tc.tile_pool(name="o", bufs=2) as opool,
            tc.tile_pool(name="at", bufs=2) as atpool,
            tc.tile_pool(name="psmm", bufs=2, space="PSUM") as psmm,
            tc.tile_pool(name="pss", bufs=2, space="PSUM") as pss,
            tc.tile_pool(name="pspv", bufs=2, space="PSUM") as pspv,
        ):
            # ---- persistent weights (outside the bench loop) ----
            wq_sb = wpool.tile([128, DT, HPC2], bf16, tag="wq")
            wk_sb = wpool.tile([128, DT, HPC2], bf16, tag="wk")
            wv_sb = wpool.tile([128, DT, HPC2], bf16, tag="wv")
            wo_sb = wpool.tile([128, NSLOT, D], bf16, tag="wo")
            bias_sb = wpool.tile([KT, NSLOT], fp32, tag="bias")
            nc.sync.dma_start(wk_sb[:], wk.ap().rearrange("(t p) c -> p t c", p=128))
            nc.sync.dma_start(wq_sb[:], wq.ap().rearrange("(t p) c -> p t c", p=128))
            nc.sync.dma_start(wv_sb[:], wv.ap().rearrange("(t p) c -> p t c", p=128))
            nc.sync.dma_start(wo_sb[:], wo.ap().rearrange("(c p) n -> p c n", p=128))
            nc.sync.dma_start(bias_sb[:], bias_in.ap())

            loop_cm = (
                tc.For_i(0, bench_iters, 1)
                if bench_iters > 0
                else contextlib.nullcontext()
            )
            with loop_cm:
                _emit_body(nc, tc, locals())

    nc.compile()
    return nc


def _emit_body(nc, tc, env):
    from collections import deque
    import concourse.mybir as mybir

    fp32 = mybir.dt.float32
    bf16 = mybir.dt.bfloat16
    fp16 = mybir.dt.float16
    EXP = mybir.ActivationFunctionType.Exp
    Ts = env["Ts"]
    DT, CKMAX = env["DT"], env["CKMAX"]
    qT, kT, vT, out2 = env["qT"], env["kT"], env["vT"], env["out2"]
    wq_sb, wk_sb, wv_sb, wo_sb = env["wq_sb"], env["wk_sb"], env["wv_sb"], env["wo_sb"]
    bias_sb = env["bias_sb"]
    xkvpool, xqpool = env["xkvpool"], env["xqpool"]
    ppool, qkpool, smpool = env["ppool"], env["qkpool"], env["smpool"]
    sm1pool, opool = env["sm1pool"], env["opool"]
    psmm, pss, pspv = env["psmm"], env["pss"], env["pspv"]

    NQ = S // QB  # 4 q blocks
    TMAX = max(Ts)
    vsort = vsort_order(Ts)

    kT_r = kT.ap().rearrange("(t p) k -> p t k", p=128)
    qT_r = qT.ap().rearrange("(t p) q -> p t q", p=128)
    vT_r = vT.ap().rearrange("(t p) k -> p t k", p=128)

    # ---- persistent SBUF staging ----
    kts = [
        qkpool.tile([128, Ts[s] * KT], bf16, tag=f"kts{s}", name=f"kts{s}")
        for s in range(NSLOT)
    ]
    qts = [
        qkpool.tile([128, S], bf16, tag=f"qts{s}", name=f"qts{s}")
        for s in range(NSLOT)
    ]
    ve = [
        qkpool.tile([128, Ts[s], 2, HD + 1], bf16, tag=f"ve{s}", name=f"ve{s}")
        for s in range(NSLOT)
    ]
    # aT is per-(qb, slot): written by normalize, consumed by wo two steps
    # later at most, so a depth-2 rotation replaces full-S staging
    atpool = env["atpool"]
    aT_cur = {}

    # ones column of V_ext (the PV matmul's 65th row = softmax denominator)
    for s in range(NSLOT):
        nc.gpsimd.memset(ve[s][:, :, :, HD : HD + 1], 1.0)

    # ---- K projection, chunk-paced (kts[s][w, k], w: [hA 64 | hB 64]) ----
    for c0 in range(0, CKMAX, KC):
        cw = min(KC, CKMAX - c0)
        xkc = xkvpool.tile([128, DT, KC], bf16, tag="xkv", name="xkc")
        nc.sync.dma_start(xkc[:, :, :cw], kT_r[:, :, c0 : c0 + cw])
        for s in range(NSLOT):
            bw = min(Ts[s] * KT - c0, cw)
            if bw <= 0:
                continue
            csl = slice(s * 128, (s + 1) * 128)
            ps = psmm.tile([128, QB], fp32, tag="mm", name="psk")
            for dt in range(DT):
                nc.tensor.matmul(
                    ps[:, :bw],
                    wk_sb[:, dt, csl],
                    xkc[:, dt, :bw],
                    start=(dt == 0),
                    stop=(dt == DT - 1),
                )
            nc.vector.tensor_copy(kts[s][:, c0 : c0 + bw], ps[:, :bw])

    # ---- Q projection for chunk c (q columns [c*KC, (c+1)*KC)) ----
    def emit_qproj(c, half=None):
        xqc = xq_tiles[c]
        if half is None:
            los = [(0, KC)]
        else:
            los = [(half * (KC // 2), KC // 2)]
        for lo, w in los:
            for s in range(NSLOT):
                csl = slice(s * 128, (s + 1) * 128)
                ps = psmm.tile([128, QB], fp32, tag="mm", name="psq")
                for dt in range(DT):
                    nc.tensor.matmul(
                        ps[:, :w],
                        wq_sb[:, dt, csl],
                        xqc[:, dt, lo : lo + w],
                        start=(dt == 0),
                        stop=(dt == DT - 1),
                    )
                nc.vector.tensor_copy(
                    qts[s][:, c * KC + lo : c * KC + lo + w], ps[:, :w]
                )

    # xq chunk DMAs: c0/c1 prefetch now, c2/c3 rotate in as c0/c1 retire
    xq_tiles = {}

    def load_xq(c):
        xqc = xqpool.tile([128, DT, KC], bf16, tag="xq", name="xqc")
        nc.sync.dma_start(xqc[:], qT_r[:, :, c * KC : (c + 1) * KC])
        xq_tiles[c] = xqc

    load_xq(0)
    load_xq(1)  # prefetch into the second xq buffer
    emit_qproj(0)

    # ---- merged V projection for one k-tile (slots in vsort order) ----
    xv_cur = [None]

    def emit_vproj_kt(kt):
        nact = sum(1 for s in vsort if Ts[s] > kt)
        width = nact * 128
        cidx = kt % (KC // KT)
        if cidx == 0:
            xvc = xkvpool.tile([128, DT, KC], bf16, tag="xkv", name="xvc")
            c0 = kt * KT
            cw = min(KC, CKMAX - c0)
            nc.sync.dma_start(xvc[:, :, :cw], vT_r[:, :, c0 : c0 + cw])
            xv_cur[0] = xvc
        xvc = xv_cur[0]
        ps = psmm.tile([128, QB], fp32, tag="mm", name="psv")
        for dt in range(DT):
            nc.tensor.matmul(
                ps[:, :width],
                xvc[:, dt, cidx * KT : (cidx + 1) * KT],
                wv_sb[:, dt, 0:width],
                start=(dt == 0),
                stop=(dt == DT - 1),
            )
        for j in range(nact):
            s = vsort[j]
            nc.vector.tensor_copy(
                ve[s][:, kt, :, 0:HD],
                ps[:, j * 128 : (j + 1) * 128].rearrange("p (h d) -> p h d", h=2),
            )

    # ---- per-(qb, s) attention pieces ----
    def emit_scores_kt(qb, s, kt, p):
        qsl = slice(qb * QB, (qb + 1) * QB)
        ksl = slice(kt * KT, (kt + 1) * KT)
        ss = pss.tile([128, 2, QB], fp32, tag="s", name="ss")
        # scores^T, 2 same-class heads as independent 64-row PE tiles
        nc.tensor.matmul(ss[:, 0, :], kts[s][0:64, ksl], qts[s][0:64, qsl])
        nc.tensor.matmul(ss[:, 1, :], kts[s][64:128, ksl], qts[s][64:128, qsl])
        bias_ap = bias_sb[:, s : s + 1] if kt == Ts[s] - 1 else 0.0
        # contiguous [128, 1024] exp write (kt-major P layout)
        nc.scalar.activation(p[:, kt, :, :], ss[:], EXP, bias=bias_ap, scale=0.125)

    pv_tiles = {}

    def emit_pv_h(qb, s, h, p):
        pv = pspv.tile([128, QB], fp32, tag="pv", name=f"pv{h}")
        pv_tiles[h] = pv
        for kt in range(Ts[s]):
            nc.tensor.matmul(
                pv[0 : HD + 1, :],
                ve[s][:, kt, h, :],
                p[:, kt, h, :],
                start=(kt == 0),
                stop=(kt == Ts[s] - 1),
            )

    def emit_norm(qb, s):
        # aT[h*64:(h+1)*64, :] = pv[h][:64] / pv[h][64]
        # (sum rows moved to partition 0 by DMA: reciprocal /
        #  partition_broadcast only work from base partition 0)
        pv = pv_tiles
        at = atpool.tile([128, QB], bf16, tag=f"at{s}", name=f"at{s}")
        aT_cur[s] = at
        s_sb = sm1pool.tile([HD + 1, 2, QB], fp32, tag="ssb", name="ssb")
        for h in range(2):
            nc.vector.tensor_copy(s_sb[HD : HD + 1, h, :], pv[h][HD : HD + 1, :])
        s0 = sm1pool.tile([1, 2, QB], fp32, tag="s0", name="s0")
        nc.sync.dma_start(s0[:], s_sb[HD : HD + 1, :, :])
        nc.vector.reciprocal(s0[:], s0[:])
        rb = smpool.tile([HD, 2, QB], fp32, tag="rb", name="rb")
        nc.gpsimd.partition_broadcast(rb[:], s0[0:1, :, :])
        nc.vector.tensor_mul(at[0:HD, :], pv[0][0:HD, :], rb[:, 0, :])
        tmp = smpool.tile([HD, QB], bf16, tag="tmp", name="tmp")
        nc.vector.tensor_mul(tmp[:], pv[1][0:HD, :], rb[:, 1, :])
        nc.sync.dma_start(at[HD:128, :], tmp[:])

    def emit_wo(qt, nh, ats):
        # out2 rows for q tile qt (fp16 SBUF staging, DVE/ACT evacuation)
        nsl = slice(nh * 512, (nh + 1) * 512)
        qoff = (qt * 128) % QB
        ps = psmm.tile([128, QB], fp32, tag="mm", name="pso")
        for s in range(NSLOT):
            nc.tensor.matmul(
                ps[:],
                ats[s][:, qoff : qoff + 128],
                wo_sb[:, s, nsl],
                start=(s == 0),
                stop=(s == NSLOT - 1),
            )
        ob = opool.tile([128, QB], fp16, tag="ob", name="ob")
        if nh == 0:
            nc.vector.tensor_copy(ob[:], ps[:])
        else:
            nc.scalar.copy(ob[:], ps[:])
        nc.sync.dma_start(out2.ap()[qt * 128 : (qt + 1) * 128, nsl], ob[:])

    # ---- pipelined emission ----
    # FIFO of (push_step, kind, fn).  Drained between score k-tiles; FIFO
    # order + forced drains keep every PE instruction's PE-side producers
    # ahead of it in the queue (in-order PE would deadlock otherwise).
    fifo = deque()
    # slots in descending-T order: the last slot of each q block is the
    # smallest, so the end-of-iteration tail behind the final exp is short
    steps = [(qb, s) for qb in range(NQ) for s in vsort]

    def drain(n=None):
        while fifo and (n is None or n > 0):
            _, _, fn = fifo.popleft()
            fn()
            if n is not None:
                n -= 1

    def drain_older_than(idx, lag):
        while fifo and fifo[0][0] <= idx - lag:
            _, _, fn = fifo.popleft()
            fn()

    def drain_until_kind_done(kind):
        """Emit FIFO entries (in order) until none of `kind` remain."""
        while any(e[1] == kind for e in fifo):
            _, _, fn = fifo.popleft()
            fn()

    for idx, (qb, s) in enumerate(steps):
        T = Ts[s]
        pos = idx % NSLOT
        # hard ordering: qts chunk qb and pv(qb-1, s) must already be emitted
        # before this step's score matmuls / exp enter the queues
        if pos == 0 and qb > 0:
            drain_until_kind_done(f"qp{qb}")
        drain_until_kind_done(f"pv{qb - 1}_{s}")
        drain_older_than(idx, 3)

        p = ppool.tile([128, T, 2, QB], bf16, tag=f"p{s}", name=f"p{s}")
        for kt in range(T):
            emit_scores_kt(qb, s, kt, p)
            if kt % 2 == 1:
                drain(2 if len(fifo) > 6 else 1)

        # queue follow-on work for this step
        if idx == 0:
            # all of V-proj ahead of the first PV (vsort[0] has T == TMAX)
            for kt in range(TMAX):
                fifo.append((idx, "vp", lambda kt=kt: emit_vproj_kt(kt)))
        pk = f"pv{qb}_{s}"
        fifo.append((idx, pk, lambda a=qb, b=s, pt=p: emit_pv_h(a, b, 0, pt)))
        fifo.append((idx, pk, lambda a=qb, b=s, pt=p: emit_pv_h(a, b, 1, pt)))
        fifo.append((idx, "nm", lambda a=qb, b=s: emit_norm(a, b)))
        if pos == 1 and qb + 1 < NQ:
            fifo.append(
                (idx, f"qp{qb + 1}", lambda c=qb + 1: emit_qproj(c, half=0))
            )
            fifo.append(
                (idx, f"qp{qb + 1}", lambda c=qb + 1: emit_qproj(c, half=1))
            )
        if pos == 2 and qb + 2 < NQ:
            fifo.append((idx, "ld", lambda c=qb + 2: load_xq(c)))
        if pos == NSLOT - 1:
            for qt in range(qb * (QB // 128), (qb + 1) * (QB // 128)):
                for nh in range(2):
                    fifo.append(
                        (idx, "wo", lambda a=qt, b=nh: emit_wo(a, b, aT_cur))
                    )

    drain()


def build_in_maps(query, key, value, valid_length, Wq, Wk, Wv, Wo):
    """Host-side sharding. Returns (Ts, in_maps)."""
    valid = np.asarray(valid_length).astype(np.int64)
    Ts = tuple(int(-(-v // KT)) for v in valid)
    CKMAX = max(Ts) * KT
    vsort = vsort_order(Ts)

    bf = ml_dtypes.bfloat16
    query = np.asarray(query)
    key = np.asarray(key)
    value = np.asarray(value)
    qTs = [np.ascontiguousarray(query[b].T).astype(bf) for b in range(B)]
    kTs = [np.ascontiguousarray(key[b].T[:, :CKMAX]).astype(bf) for b in range(B)]
    vTs = [np.ascontiguousarray(value[b].T[:, :CKMAX]).astype(bf) for b in range(B)]

    bias = np.zeros((KT, NSLOT), np.float32)
    for s in range(NSLOT):
        rem = int(valid[s]) - (Ts[s] - 1) * KT  # 1..128 valid rows in last tile
        bias[rem:, s] = MASK_BIAS

    Wqb = np.asarray(Wq).astype(bf)
    Wkb = np.asarray(Wk).astype(bf)
    Wvb = np.asarray(Wv).astype(bf)
    Wob = np.asarray(Wo).astype(bf)

    in_maps = []
    for c in range(NCORES):
        beta = c % 4
        heads = core_heads(c)
        hcols = np.concatenate(
            [np.arange(h * HD, (h + 1) * HD) for h in heads]
        )
        # wv columns in vsort slot order (merged V-proj reads a prefix)
        vcols = np.concatenate(
            [
                np.concatenate(
                    [
                        np.arange(heads[2 * s] * HD, (heads[2 * s] + 1) * HD),
                        np.arange(
                            heads[2 * s + 1] * HD, (heads[2 * s + 1] + 1) * HD
                        ),
                    ]
                )
                for s in vsort
            ]
        )
        in_maps.append(
            {
                "qT": qTs[beta],
                "kT": kTs[beta],
                "vT": vTs[beta],
                "wq": np.ascontiguousarray(Wqb[:, hcols]),
                "wk": np.ascontiguousarray(Wkb[:, hcols]),
                "wv": np.ascontiguousarray(Wvb[:, vcols]),
                "wo": np.ascontiguousarray(Wob[hcols, :]),
                "bias": bias,
            }
        )
    return Ts, in_maps


def kernel(query, key, value, valid_length, Wq, Wk, Wv, Wo):
    from concourse.bass_utils import run_bass_kernel_spmd

    Ts, in_maps = build_in_maps(
        query, key, value, valid_length, Wq, Wk, Wv, Wo
    )
    if Ts not in _compiled:
        _compiled[Ts] = _build(Ts)
    nc = _compiled[Ts]

    res = run_bass_kernel_spmd(nc, in_maps, list(range(NCORES)))
    out = np.zeros((B, S, D), np.float32)
    for c in range(NCORES):
        out[c % 4] += res.results[c]["out2"].astype(np.float32)
    return out


# revision 26
# speedup vs baseline: 1.0032x; 1.0032x over previous
"""Multi-head attention (B=4, S=2048, D=1024, H=16) on 8 NeuronCores.

Reference quirk: the key-padding mask uses jnp.tile(valid_length, H) indexed
by the flat (b*H + h) head-batch index, so the effective mask length for
(batch b, head h) is valid_length[h % 4] -- it depends on the head CLASS
(h mod 4), not the batch.

Sharding: core i handles batch i%4 and the 8 heads {4P..4P+3, 4P+8..4P+11}
(P = i//4).  Those 8 heads contain each mask class exactly twice, so every
core does identical work (load-balanced by construction), and key/value work
beyond valid_length[class] (rounded up to 128) is skipped entirely.  The two
same-class heads (h, h+8) are row-packed into one 64-contraction PE pair.
Per-core partial outputs (rank-512 contributions through Wo) are summed on
the host (cores i and i+4 hold the two halves of batch i%4's heads).

All matmuls run in bf16 (fp32 PSUM accumulation).  Attention is computed in
"transposed" orientation S^T[k, q] so that softmax masking is a per-partition
exp bias, the k-sum comes free via an appended ones-column on V, and no
on-chip transposes are needed anywhere.

The emission is software-pipelined around the exp bottleneck: the ACT engine
(the only engine that can do exp) needs ~1.15us per 128-key tile while the
PE produces that tile's scores in ~0.43us, so between score tiles the PE is
fed "filler" work -- PV accumulation of the previous (qb, slot) step, the
merged V projection, late Q-projection chunks, and Wo output tiles -- from a
FIFO whose order also guarantees the in-order PE queue can never wait on an
instruction queued behind it.  Wo results stream straight from PSUM to DRAM
in fp32 (no staging copy).
"""

import sys

for _p in ("/opt/trn_rl_repo", "/root/.axon_site/_ro/trn_rl_repo"):
    if _p not in sys.path:
        sys.path.insert(0, _p)

import numpy as np
import ml_dtypes

B, S, D, H = 4, 2048, 1024, 16
HD = D // H  # 64
NCORES = 8
NSLOT = 4  # head classes (h % 4) per core, 2 heads each
KT = 128  # k-tile size
QB = 512  # q block
KC = 512  # k/q DMA chunk width
MASK_BIAS = -30000.0  # exp(s/8 + bias) == 0 for masked rows (s/8 is O(10))

_compiled = {}  # (T0,T1,T2,T3) -> compiled nc


def core_heads(core):
    """The 8 heads of `core`, in (slot, pair) order: [hA0, hB0, hA1, ...]."""
    P = core // 4
    heads = []
    for c in range(NSLOT):
        heads += [c + 4 * P, c + 8 + 4 * P]
    return heads


def vsort_order(Ts):
    """Slots sorted by descending k-tile count (ties by slot index)."""
    return sorted(range(NSLOT), key=lambda s: (-Ts[s], s))


def _build(Ts, bench_iters=0):
    """Build + compile the single SPMD program for k-tile class profile Ts.

    bench_iters > 0 wraps the whole body in a hardware loop for timing.
    """
    import contextlib
    import concourse.bacc as bacc
    import concourse.tile as tile
    import concourse.mybir as mybir

    fp32 = mybir.dt.float32
    bf16 = mybir.dt.bfloat16

    CKMAX = max(Ts) * KT
    DT = D // 128  # 8 contraction tiles for the projections
    HPC2 = 2 * NSLOT * HD  # 512 head-dim columns per core

    nc = bacc.Bacc("TRN2", target_bir_lowering=False, debug=False, num_devices=NCORES)

    qT = nc.dram_tensor("qT", [D, S], bf16, kind="ExternalInput")
    kT = nc.dram_tensor("kT", [D, CKMAX], bf16, kind="ExternalInput")
    vT = nc.dram_tensor("vT", [D, CKMAX], bf16, kind="ExternalInput")
    wq = nc.dram_tensor("wq", [D, HPC2], bf16, kind="ExternalInput")
    wk = nc.dram_tensor("wk", [D, HPC2], bf16, kind="ExternalInput")
    wv = nc.dram_tensor("wv", [D, HPC2], bf16, kind="ExternalInput")
    wo = nc.dram_tensor("wo", [HPC2, D], bf16, kind="ExternalInput")
    bias_in = nc.dram_tensor("bias", [KT, NSLOT], fp32, kind="ExternalInput")
    fp16 = mybir.dt.float16
    out2 = nc.dram_tensor("out2", [S, D], fp16, kind="ExternalOutput")

    with tile.TileContext(nc) as tc:
        with (
            tc.tile_pool(name="w", bufs=1) as wpool,
            tc.tile_pool(name="xkv", bufs=2) as xkvpool,
            tc.tile_pool(name="xq", bufs=2) as xqpool,
            tc.tile_pool(name="p", bufs=1) as ppool,
            tc.tile_pool(name="qk", bufs=1) as qkpool,
            tc.tile_pool(name="sm", bufs=2) as smpool,
            tc.tile_pool(name="sm1", bufs=1) as sm1pool,
            tc.tile_pool(name="o", bufs=2) as opool,
            tc.tile_pool(name="at", bufs=2) as atpool,
            tc.tile_pool(name="psmm", bufs=2, space="PSUM") as psmm,
            tc.tile_pool(name="pss", bufs=2, space="PSUM") as pss,
            tc.tile_pool(name="pspv", bufs=2, space="PSUM") as pspv,
        ):
            # ---- persistent weights (outside the bench loop) ----
            wq_sb = wpool.tile([128, DT, HPC2], bf16, tag="wq")
            wk_sb = wpool.tile([128, DT, HPC2], bf16, tag="wk")
            wv_sb = wpool.tile([128, DT, HPC2], bf16, tag="wv")
            wo_sb = wpool.tile([128, NSLOT, D], bf16, tag="wo")
            bias_sb = wpool.tile([KT, NSLOT], fp32, tag="bias")
            nc.sync.dma_start(wk_sb[:], wk.ap().rearrange("(t p) c -> p t c", p=128))
            nc.sync.dma_start(wq_sb[:], wq.ap().rearrange("(t p) c -> p t c", p=128))
            nc.sync.dma_start(wv_sb[:], wv.ap().rearrange("(t p) c -> p t c", p=128))
            nc.sync.dma_start(wo_sb[:], wo.ap().rearrange("(c p) n -> p c n", p=128))
            nc.sync.dma_start(bias_sb[:], bias_in.ap())

            loop_cm = (
                tc.For_i(0, bench_iters, 1)
                if bench_iters > 0
                else contextlib.nullcontext()
            )
            with loop_cm:
                _emit_body(nc, tc, locals())

    nc.compile()
    return nc


def _emit_body(nc, tc, env):
    from collections import deque
    import concourse.mybir as mybir

    fp32 = mybir.dt.float32
    bf16 = mybir.dt.bfloat16
    fp16 = mybir.dt.float16
    EXP = mybir.ActivationFunctionType.Exp
    Ts = env["Ts"]
    DT, CKMAX = env["DT"], env["CKMAX"]
    qT, kT, vT, out2 = env["qT"], env["kT"], env["vT"], env["out2"]
    wq_sb, wk_sb, wv_sb, wo_sb = env["wq_sb"], env["wk_sb"], env["wv_sb"], env["wo_sb"]
    bias_sb = env["bias_sb"]
    xkvpool, xqpool = env["xkvpool"], env["xqpool"]
    ppool, qkpool, smpool = env["ppool"], env["qkpool"], env["smpool"]
    sm1pool, opool = env["sm1pool"], env["opool"]
    psmm, pss, pspv = env["psmm"], env["pss"], env["pspv"]

    NQ = S // QB  # 4 q blocks
    TMAX = max(Ts)
    vsort = vsort_order(Ts)

    kT_r = kT.ap().rearrange("(t p) k -> p t k", p=128)
    qT_r = qT.ap().rearrange("(t p) q -> p t q", p=128)
    vT_r = vT.ap().rearrange("(t p) k -> p t k", p=128)

    # ---- persistent SBUF staging ----
    kts = [
        qkpool.tile([128, Ts[s] * KT], bf16, tag=f"kts{s}", name=f"kts{s}")
        for s in range(NSLOT)
    ]
    qts = [
        qkpool.tile([128, S], bf16, tag=f"qts{s}", name=f"qts{s}")
        for s in range(NSLOT)
    ]
    ve = [
        qkpool.tile([128, Ts[s], 2, HD + 1], bf16, tag=f"ve{s}", name=f"ve{s}")
        for s in range(NSLOT)
    ]
    # aT is per-(qb, slot): written by normalize, consumed by wo two steps
    # later at most, so a depth-2 rotation replaces full-S staging
    atpool = env["atpool"]
    aT_cur = {}

    # ones column of V_ext (the PV matmul's 65th row = softmax denominator)
    for s in range(NSLOT):
        nc.gpsimd.memset(ve[s][:, :, :, HD : HD + 1], 1.0)

    # ---- K projection, chunk-paced (kts[s][w, k], w: [hA 64 | hB 64]) ----
    for c0 in range(0, CKMAX, KC):
        cw = min(KC, CKMAX - c0)
        xkc = xkvpool.tile([128, DT, KC], bf16, tag="xkv", name="xkc")
        nc.sync.dma_start(xkc[:, :, :cw], kT_r[:, :, c0 : c0 + cw])
        for s in range(NSLOT):
            bw = min(Ts[s] * KT - c0, cw)
            if bw <= 0:
                continue
            csl = slice(s * 128, (s + 1) * 128)
            ps = psmm.tile([128, QB], fp32, tag="mm", name="psk")
            for dt in range(DT):
                nc.tensor.matmul(
                    ps[:, :bw],
                    wk_sb[:, dt, csl],
                    xkc[:, dt, :bw],
                    start=(dt == 0),
                    stop=(dt == DT - 1),
                )
            nc.vector.tensor_copy(kts[s][:, c0 : c0 + bw], ps[:, :bw])

    # ---- Q projection for chunk c (q columns [c*KC, (c+1)*KC)) ----
    def emit_qproj(c, half=None):
        xqc = xq_tiles[c]
        if half is None:
            los = [(0, KC)]
        else:
            los = [(half * (KC // 2), KC // 2)]
        for lo, w in los:
            for s in range(NSLOT):
                csl = slice(s * 128, (s + 1) * 128)
                ps = psmm.tile([128, QB], fp32, tag="mm", name="psq")
                for dt in range(DT):
                    nc.tensor.matmul(
                        ps[:, :w],
                        wq_sb[:, dt, csl],
                        xqc[:, dt, lo : lo + w],
                        start=(dt == 0),
                        stop=(dt == DT - 1),
                    )
                nc.vector.tensor_copy(
                    qts[s][:, c * KC + lo : c * KC + lo + w], ps[:, :w]
                )

    # xq chunk DMAs: c0/c1 prefetch now, c2/c3 rotate in as c0/c1 retire
    xq_tiles = {}

    def load_xq(c):
        xqc = xqpool.tile([128, DT, KC], bf16, tag="xq", name="xqc")
        nc.sync.dma_start(xqc[:], qT_r[:, :, c * KC : (c + 1) * KC])
        xq_tiles[c] = xqc

    load_xq(0)
    load_xq(1)  # prefetch into the second xq buffer
    emit_qproj(0)

    # ---- merged V projection for one k-tile (slots in vsort order) ----
    xv_cur = [None]

    def emit_vproj_kt(kt):
        nact = sum(1 for s in vsort if Ts[s] > kt)
        width = nact * 128
        cidx = kt % (KC // KT)
        if cidx == 0:
            xvc = xkvpool.tile([128, DT, KC], bf16, tag="xkv", name="xvc")
            c0 = kt * KT
            cw = min(KC, CKMAX - c0)
            nc.sync.dma_start(xvc[:, :, :cw], vT_r[:, :, c0 : c0 + cw])
            xv_cur[0] = xvc
        xvc = xv_cur[0]
        ps = psmm.tile([128, QB], fp32, tag="mm", name="psv")
        for dt in range(DT):
            nc.tensor.matmul(
                ps[:, :width],
                xvc[:, dt, cidx * KT : (cidx + 1) * KT],
                wv_sb[:, dt, 0:width],
                start=(dt == 0),
                stop=(dt == DT - 1),
            )
        for j in range(nact):
            s = vsort[j]
            nc.vector.tensor_copy(
                ve[s][:, kt, :, 0:HD],
                ps[:, j * 128 : (j + 1) * 128].rearrange("p (h d) -> p h d", h=2),
            )

    # ---- per-(qb, s) attention pieces ----
    def emit_scores_kt(qb, s, kt, p):
        qsl = slice(qb * QB, (qb + 1) * QB)
        ksl = slice(kt * KT, (kt + 1) * KT)
        ss = pss.tile([128, 2, QB], fp32, tag="s", name="ss")
        # scores^T, 2 same-class heads as independent 64-row PE tiles
        nc.tensor.matmul(ss[:, 0, :], kts[s][0:64, ksl], qts[s][0:64, qsl])
        nc.tensor.matmul(ss[:, 1, :], kts[s][64:128, ksl], qts[s][64:128, qsl])
        bias_ap = bias_sb[:, s : s + 1] if kt == Ts[s] - 1 else 0.0
        # contiguous [128, 1024] exp write (kt-major P layout)
        nc.scalar.activation(p[:, kt, :, :], ss[:], EXP, bias=bias_ap, scale=0.125)

    pv_tiles = {}

    def emit_pv_h(qb, s, h, p):
        pv = pspv.tile([128, QB], fp32, tag="pv", name=f"pv{h}")
        pv_tiles[h] = pv
        for kt in range(Ts[s]):
            nc.tensor.matmul(
                pv[0 : HD + 1, :],
                ve[s][:, kt, h, :],
                p[:, kt, h, :],
                start=(kt == 0),
                stop=(kt == Ts[s] - 1),
            )

    def emit_norm(qb, s):
        # aT[h*64:(h+1)*64, :] = pv[h][:64] / pv[h][64]
        # (sum rows moved to partition 0 by DMA: reciprocal /
        #  partition_broadcast only work from base partition 0)
        pv = pv_tiles
        at = atpool.tile([128, QB], bf16, tag=f"at{s}", name=f"at{s}")
        aT_cur[s] = at
        s_sb = sm1pool.tile([HD + 1, 2, QB], fp32, tag="ssb", name="ssb")
        for h in range(2):
            nc.vector.tensor_copy(s_sb[HD : HD + 1, h, :], pv[h][HD : HD + 1, :])
        s0 = sm1pool.tile([1, 2, QB], fp32, tag="s0", name="s0")
        nc.sync.dma_start(s0[:], s_sb[HD : HD + 1, :, :])
        nc.vector.reciprocal(s0[:], s0[:])
        rb = smpool.tile([HD, 2, QB], fp32, tag="rb", name="rb")
        nc.gpsimd.partition_broadcast(rb[:], s0[0:1, :, :])
        nc.vector.tensor_mul(at[0:HD, :], pv[0][0:HD, :], rb[:, 0, :])
        tmp = smpool.tile([HD, QB], bf16, tag="tmp", name="tmp")
        nc.vector.tensor_mul(tmp[:], pv[1][0:HD, :], rb[:, 1, :])
        nc.sync.dma_start(at[HD:128, :], tmp[:])

    def emit_wo(qt, nh, ats):
        # out2 rows for q tile qt (fp16 SBUF staging, DVE/ACT evacuation)
        nsl = slice(nh * 512, (nh + 1) * 512)
        qoff = (qt * 128) % QB
        ps = psmm.tile([128, QB], fp32, tag="mm", name="pso")
        for s in range(NSLOT):
            nc.tensor.matmul(
                ps[:],
                ats[s][:, qoff : qoff + 128],
                wo_sb[:, s, nsl],
                start=(s == 0),
                stop=(s == NSLOT - 1),
            )
        ob = opool.tile([128, QB], fp16, tag="ob", name="ob")
        nc.vector.tensor_copy(ob[:], ps[:])
        nc.sync.dma_start(out2.ap()[qt * 128 : (qt + 1) * 128, nsl], ob[:])

    # ---- pipelined emission ----
    # FIFO of (push_step, kind, fn).  Drained between score k-tiles; FIFO
    # order + forced drains keep every PE instruction's PE-side producers
    # ahead of it in the queue (in-order PE would deadlock otherwise).
    fifo = deque()
    # slots in descending-T order: the last slot of each q block is the
    # smallest, so the end-of-iteration tail behind the final exp is short
    steps = [(qb, s) for qb in range(NQ) for s in vsort]

    def drain(n=None):
        while fifo and (n is None or n > 0):
            _, _, fn = fifo.popleft()
            fn()
            if n is not None:
                n -= 1

    def drain_older_than(idx, lag):
        while fifo and fifo[0][0] <= idx - lag:
            _, _, fn = fifo.popleft()
            fn()

    def drain_until_kind_done(kind):
        """Emit FIFO entries (in order) until none of `kind` remain."""
        while any(e[1] == kind for e in fifo):
            _, _, fn = fifo.popleft()
            fn()

    for idx, (qb, s) in enumerate(steps):
        T = Ts[s]
        pos = idx % NSLOT
        # hard ordering: qts chunk qb and pv(qb-1, s) must already be emitted
        # before this step's score matmuls / exp enter the queues
        if pos == 0 and qb > 0:
            drain_until_kind_done(f"qp{qb}")
        drain_until_kind_done(f"pv{qb - 1}_{s}")
        drain_older_than(idx, 3)

        p = ppool.tile([128, T, 2, QB], bf16, tag=f"p{s}", name=f"p{s}")
        for kt in range(T):
            emit_scores_kt(qb, s, kt, p)
            if kt % 2 == 1:
                drain(2 if len(fifo) > 6 else 1)

        # queue follow-on work for this step
        if idx == 0:
            # all of V-proj ahead of the first PV (vsort[0] has T == TMAX)
            for kt in range(TMAX):
                fifo.append((idx, "vp", lambda kt=kt: emit_vproj_kt(kt)))
        pk = f"pv{qb}_{s}"
        fifo.append((idx, pk, lambda a=qb, b=s, pt=p: emit_pv_h(a, b, 0, pt)))
        fifo.append((idx, pk, lambda a=qb, b=s, pt=p: emit_pv_h(a, b, 1, pt)))
        fifo.append((idx, "nm", lambda a=qb, b=s: emit_norm(a, b)))
        if pos == 1 and qb + 1 < NQ:
            fifo.append((idx, f"qp{qb + 1}", lambda c=qb + 1: emit_qproj(c)))
        if pos == 2 and qb + 2 < NQ:
            fifo.append((idx, "ld", lambda c=qb + 2: load_xq(c)))
        if pos == NSLOT - 1:
            for qt in range(qb * (QB // 128), (qb + 1) * (QB // 128)):
                for nh in range(2):
                    fifo.append(
                        (idx, "wo", lambda a=qt, b=nh: emit_wo(a, b, aT_cur))
                    )

    drain()


def build_in_maps(query, key, value, valid_length, Wq, Wk, Wv, Wo):
    """Host-side sharding. Returns (Ts, in_maps)."""
    valid = np.asarray(valid_length).astype(np.int64)
    Ts = tuple(int(-(-v // KT)) for v in valid)
    CKMAX = max(Ts) * KT
    vsort = vsort_order(Ts)

    bf = ml_dtypes.bfloat16
    query = np.asarray(query)
    key = np.asarray(key)
    value = np.asarray(value)
    qTs = [np.ascontiguousarray(query[b].T).astype(bf) for b in range(B)]
    kTs = [np.ascontiguousarray(key[b].T[:, :CKMAX]).astype(bf) for b in range(B)]
    vTs = [np.ascontiguousarray(value[b].T[:, :CKMAX]).astype(bf) for b in range(B)]

    bias = np.zeros((KT, NSLOT), np.float32)
    for s in range(NSLOT):
        rem = int(valid[s]) - (Ts[s] - 1) * KT  # 1..128 valid rows in last tile
        bias[rem:, s] = MASK_BIAS

    Wqb = np.asarray(Wq).astype(bf)
    Wkb = np.asarray(Wk).astype(bf)
    Wvb = np.asarray(Wv).astype(bf)
    Wob = np.asarray(Wo).astype(bf)

    in_maps = []
    for c in range(NCORES):
        beta = c % 4
        heads = core_heads(c)
        hcols = np.concatenate(
            [np.arange(h * HD, (h + 1) * HD) for h in heads]
        )
        # wv columns in vsort slot order (merged V-proj reads a prefix)
        vcols = np.concatenate(
            [
                np.concatenate(
                    [
                        np.arange(heads[2 * s] * HD, (heads[2 * s] + 1) * HD),
                        np.arange(
                            heads[2 * s + 1] * HD, (heads[2 * s + 1] + 1) * HD
                        ),
                    ]
                )
                for s in vsort
            ]
        )
        in_maps.append(
            {
                "qT": qTs[beta],
                "kT": kTs[beta],
                "vT": vTs[beta],
                "wq": np.ascontiguousarray(Wqb[:, hcols]),
                "wk": np.ascontiguousarray(Wkb[:, hcols]),
                "wv": np.ascontiguousarray(Wvb[:, vcols]),
                "wo": np.ascontiguousarray(Wob[hcols, :]),
                "bias": bias,
            }
        )
    return Ts, in_maps


def kernel(query, key, value, valid_length, Wq, Wk, Wv, Wo):
    from concourse.bass_utils import run_bass_kernel_spmd

    Ts, in_maps = build_in_maps(
        query, key, value, valid_length, Wq, Wk, Wv, Wo
    )
    if Ts not in _compiled:
        _compiled[Ts] = _build(Ts)
    nc = _compiled[Ts]

    res = run_bass_kernel_spmd(nc, in_maps, list(range(NCORES)))
    out = np.zeros((B, S, D), np.float32)
    for c in range(NCORES):
        out[c % 4] += res.results[c]["out2"].astype(np.float32)
    return out


# revision 27
# speedup vs baseline: 1.0403x; 1.0369x over previous
"""Multi-head attention (B=4, S=2048, D=1024, H=16) on 8 NeuronCores.

Reference quirk: the key-padding mask uses jnp.tile(valid_length, H) indexed
by the flat (b*H + h) head-batch index, so the effective mask length for
(batch b, head h) is valid_length[h % 4] -- it depends on the head CLASS
(h mod 4), not the batch.

Sharding: core i handles batch i%4 and the 8 heads {4P..4P+3, 4P+8..4P+11}
(P = i//4).  Those 8 heads contain each mask class exactly twice, so every
core does identical work (load-balanced by construction), and key/value work
beyond valid_length[class] (rounded up to 128) is skipped entirely.  The two
same-class heads (h, h+8) are row-packed into one 64-contraction PE pair.
Per-core partial outputs (rank-512 contributions through Wo) are summed on
the host (cores i and i+4 hold the two halves of batch i%4's heads).

All matmuls run in bf16 (fp32 PSUM accumulation).  Attention is computed in
"transposed" orientation S^T[k, q] so that softmax masking is a per-partition
exp bias, the k-sum comes free via an appended ones-column on V, and no
on-chip transposes are needed anywhere.

The emission is software-pipelined around the exp bottleneck: the ACT engine
(the only engine that can do exp) needs ~1.15us per 128-key tile while the
PE produces that tile's scores in ~0.43us, so between score tiles the PE is
fed "filler" work -- PV accumulation of the previous (qb, slot) step, the
merged V projection, late Q-projection chunks, and Wo output tiles -- from a
FIFO whose order also guarantees the in-order PE queue can never wait on an
instruction queued behind it.  Wo results stream straight from PSUM to DRAM
in fp32 (no staging copy).
"""

import sys

for _p in ("/opt/trn_rl_repo", "/root/.axon_site/_ro/trn_rl_repo"):
    if _p not in sys.path:
        sys.path.insert(0, _p)

import numpy as np
import ml_dtypes

B, S, D, H = 4, 2048, 1024, 16
HD = D // H  # 64
NCORES = 8
NSLOT = 4  # head classes (h % 4) per core, 2 heads each
KT = 128  # k-tile size
QB = 512  # q block
KC = 512  # k/q DMA chunk width
MASK_BIAS = -30000.0  # exp(s/8 + bias) == 0 for masked rows (s/8 is O(10))

_compiled = {}  # (T0,T1,T2,T3) -> compiled nc


def core_heads(core):
    """The 8 heads of `core`, in (slot, pair) order: [hA0, hB0, hA1, ...]."""
    P = core // 4
    heads = []
    for c in range(NSLOT):
        heads += [c + 4 * P, c + 8 + 4 * P]
    return heads


def vsort_order(Ts):
    """Slots sorted by descending k-tile count (ties by slot index)."""
    return sorted(range(NSLOT), key=lambda s: (-Ts[s], s))


def _build(Ts, bench_iters=0):
    """Build + compile the single SPMD program for k-tile class profile Ts.

    bench_iters > 0 wraps the whole body in a hardware loop for timing.
    """
    import contextlib
    import concourse.bacc as bacc
    import concourse.tile as tile
    import concourse.mybir as mybir

    fp32 = mybir.dt.float32
    bf16 = mybir.dt.bfloat16

    CKMAX = max(Ts) * KT
    DT = D // 128  # 8 contraction tiles for the projections
    HPC2 = 2 * NSLOT * HD  # 512 head-dim columns per core

    nc = bacc.Bacc("TRN2", target_bir_lowering=False, debug=False, num_devices=NCORES)

    qT = nc.dram_tensor("qT", [D, S], bf16, kind="ExternalInput")
    kT = nc.dram_tensor("kT", [D, CKMAX], bf16, kind="ExternalInput")
    vT = nc.dram_tensor("vT", [D, CKMAX], bf16, kind="ExternalInput")
    wq = nc.dram_tensor("wq", [D, HPC2], bf16, kind="ExternalInput")
    wk = nc.dram_tensor("wk", [D, HPC2], bf16, kind="ExternalInput")
    wv = nc.dram_tensor("wv", [D, HPC2], bf16, kind="ExternalInput")
    wo = nc.dram_tensor("wo", [HPC2, D], bf16, kind="ExternalInput")
    bias_in = nc.dram_tensor("bias", [KT, NSLOT], fp32, kind="ExternalInput")
    fp16 = mybir.dt.float16
    out2 = nc.dram_tensor("out2", [S, D], fp16, kind="ExternalOutput")

    with tile.TileContext(nc) as tc:
        with (
            tc.tile_pool(name="w", bufs=1) as wpool,
            tc.tile_pool(name="xkv", bufs=2) as xkvpool,
            tc.tile_pool(name="xq", bufs=2) as xqpool,
            tc.tile_pool(name="p", bufs=1) as ppool,
            tc.tile_pool(name="qk", bufs=1) as qkpool,
            tc.tile_pool(name="sm", bufs=2) as smpool,
            tc.tile_pool(name="sm1", bufs=1) as sm1pool,
            tc.tile_pool(name="o", bufs=2) as opool,
            tc.tile_pool(name="at", bufs=2) as atpool,
            tc.tile_pool(name="psmm", bufs=2, space="PSUM") as psmm,
            tc.tile_pool(name="pss", bufs=2, space="PSUM") as pss,
            tc.tile_pool(name="pspv", bufs=2, space="PSUM") as pspv,
        ):
            # ---- persistent weights (outside the bench loop) ----
            wq_sb = wpool.tile([128, DT, HPC2], bf16, tag="wq")
            wk_sb = wpool.tile([128, DT, HPC2], bf16, tag="wk")
            wv_sb = wpool.tile([128, DT, HPC2], bf16, tag="wv")
            wo_sb = wpool.tile([128, NSLOT, D], bf16, tag="wo")
            bias_sb = wpool.tile([KT, NSLOT], fp32, tag="bias")
            nc.sync.dma_start(wk_sb[:], wk.ap().rearrange("(t p) c -> p t c", p=128))
            nc.sync.dma_start(wq_sb[:], wq.ap().rearrange("(t p) c -> p t c", p=128))
            nc.sync.dma_start(wv_sb[:], wv.ap().rearrange("(t p) c -> p t c", p=128))
            nc.sync.dma_start(wo_sb[:], wo.ap().rearrange("(c p) n -> p c n", p=128))
            nc.sync.dma_start(bias_sb[:], bias_in.ap())

            loop_cm = (
                tc.For_i(0, bench_iters, 1)
                if bench_iters > 0
                else contextlib.nullcontext()
            )
            with loop_cm:
                _emit_body(nc, tc, locals())

    nc.compile()
    return nc


def _emit_body(nc, tc, env):
    from collections import deque
    import concourse.mybir as mybir

    fp32 = mybir.dt.float32
    bf16 = mybir.dt.bfloat16
    fp16 = mybir.dt.float16
    EXP = mybir.ActivationFunctionType.Exp
    Ts = env["Ts"]
    DT, CKMAX = env["DT"], env["CKMAX"]
    qT, kT, vT, out2 = env["qT"], env["kT"], env["vT"], env["out2"]
    wq_sb, wk_sb, wv_sb, wo_sb = env["wq_sb"], env["wk_sb"], env["wv_sb"], env["wo_sb"]
    bias_sb = env["bias_sb"]
    xkvpool, xqpool = env["xkvpool"], env["xqpool"]
    ppool, qkpool, smpool = env["ppool"], env["qkpool"], env["smpool"]
    sm1pool, opool = env["sm1pool"], env["opool"]
    psmm, pss, pspv = env["psmm"], env["pss"], env["pspv"]

    NQ = S // QB  # 4 q blocks
    TMAX = max(Ts)
    vsort = vsort_order(Ts)

    kT_r = kT.ap().rearrange("(t p) k -> p t k", p=128)
    qT_r = qT.ap().rearrange("(t p) q -> p t q", p=128)
    vT_r = vT.ap().rearrange("(t p) k -> p t k", p=128)

    # ---- persistent SBUF staging ----
    kts = [
        qkpool.tile([128, Ts[s] * KT], bf16, tag=f"kts{s}", name=f"kts{s}")
        for s in range(NSLOT)
    ]
    qts = [
        qkpool.tile([128, S], bf16, tag=f"qts{s}", name=f"qts{s}")
        for s in range(NSLOT)
    ]
    ve = [
        qkpool.tile([128, Ts[s], 2, HD + 1], bf16, tag=f"ve{s}", name=f"ve{s}")
        for s in range(NSLOT)
    ]
    # aT is per-(qb, slot): written by normalize, consumed by wo two steps
    # later at most, so a depth-2 rotation replaces full-S staging
    atpool = env["atpool"]
    aT_cur = {}

    # ones column of V_ext (the PV matmul's 65th row = softmax denominator)
    for s in range(NSLOT):
        nc.gpsimd.memset(ve[s][:, :, :, HD : HD + 1], 1.0)

    # ---- K projection, chunk-paced (kts[s][w, k], w: [hA 64 | hB 64]) ----
    for c0 in range(0, CKMAX, KC):
        cw = min(KC, CKMAX - c0)
        xkc = xkvpool.tile([128, DT, KC], bf16, tag="xkv", name="xkc")
        nc.sync.dma_start(xkc[:, :, :cw], kT_r[:, :, c0 : c0 + cw])
        for s in range(NSLOT):
            bw = min(Ts[s] * KT - c0, cw)
            if bw <= 0:
                continue
            csl = slice(s * 128, (s + 1) * 128)
            ps = psmm.tile([128, QB], fp32, tag="mm", name="psk")
            for dt in range(DT):
                nc.tensor.matmul(
                    ps[:, :bw],
                    wk_sb[:, dt, csl],
                    xkc[:, dt, :bw],
                    start=(dt == 0),
                    stop=(dt == DT - 1),
                )
            nc.vector.tensor_copy(kts[s][:, c0 : c0 + bw], ps[:, :bw])

    # ---- Q projection for chunk c (q columns [c*KC, (c+1)*KC)) ----
    def emit_qproj(c, half=None):
        xqc = xq_tiles[c]
        if half is None:
            los = [(0, KC)]
        else:
            los = [(half * (KC // 2), KC // 2)]
        for lo, w in los:
            for s in range(NSLOT):
                csl = slice(s * 128, (s + 1) * 128)
                ps = psmm.tile([128, QB], fp32, tag="mm", name="psq")
                for dt in range(DT):
                    nc.tensor.matmul(
                        ps[:, :w],
                        wq_sb[:, dt, csl],
                        xqc[:, dt, lo : lo + w],
                        start=(dt == 0),
                        stop=(dt == DT - 1),
                    )
                nc.vector.tensor_copy(
                    qts[s][:, c * KC + lo : c * KC + lo + w], ps[:, :w]
                )

    # xq chunk DMAs: c0/c1 prefetch now, c2/c3 rotate in as c0/c1 retire
    xq_tiles = {}

    def load_xq(c):
        xqc = xqpool.tile([128, DT, KC], bf16, tag="xq", name="xqc")
        nc.sync.dma_start(xqc[:], qT_r[:, :, c * KC : (c + 1) * KC])
        xq_tiles[c] = xqc

    load_xq(0)
    load_xq(1)  # prefetch into the second xq buffer
    emit_qproj(0)

    # ---- merged V projection for one k-tile (slots in vsort order) ----
    xv_cur = [None]

    def emit_vproj_kt(kt):
        nact = sum(1 for s in vsort if Ts[s] > kt)
        width = nact * 128
        cidx = kt % (KC // KT)
        if cidx == 0:
            xvc = xkvpool.tile([128, DT, KC], bf16, tag="xkv", name="xvc")
            c0 = kt * KT
            cw = min(KC, CKMAX - c0)
            nc.sync.dma_start(xvc[:, :, :cw], vT_r[:, :, c0 : c0 + cw])
            xv_cur[0] = xvc
        xvc = xv_cur[0]
        ps = psmm.tile([128, QB], fp32, tag="mm", name="psv")
        for dt in range(DT):
            nc.tensor.matmul(
                ps[:, :width],
                xvc[:, dt, cidx * KT : (cidx + 1) * KT],
                wv_sb[:, dt, 0:width],
                start=(dt == 0),
                stop=(dt == DT - 1),
            )
        for j in range(nact):
            s = vsort[j]
            nc.vector.tensor_copy(
                ve[s][:, kt, :, 0:HD],
                ps[:, j * 128 : (j + 1) * 128].rearrange("p (h d) -> p h d", h=2),
            )

    # ---- per-(qb, s) attention pieces ----
    def emit_scores_kt(qb, s, kt, p):
        qsl = slice(qb * QB, (qb + 1) * QB)
        ksl = slice(kt * KT, (kt + 1) * KT)
        ss = pss.tile([128, 2, QB], fp32, tag="s", name="ss")
        # scores^T, 2 same-class heads as independent 64-row PE tiles
        nc.tensor.matmul(ss[:, 0, :], kts[s][0:64, ksl], qts[s][0:64, qsl])
        nc.tensor.matmul(ss[:, 1, :], kts[s][64:128, ksl], qts[s][64:128, qsl])
        bias_ap = bias_sb[:, s : s + 1] if kt == Ts[s] - 1 else 0.0
        # contiguous [128, 1024] exp write (kt-major P layout)
        nc.scalar.activation(p[:, kt, :, :], ss[:], EXP, bias=bias_ap, scale=0.125)

    pv_tiles = {}

    def emit_pv_h(qb, s, h, p):
        pv = pspv.tile([128, QB], fp32, tag="pv", name=f"pv{h}")
        pv_tiles[h] = pv
        for kt in range(Ts[s]):
            nc.tensor.matmul(
                pv[0 : HD + 1, :],
                ve[s][:, kt, h, :],
                p[:, kt, h, :],
                start=(kt == 0),
                stop=(kt == Ts[s] - 1),
            )

    def emit_norm(qb, s):
        # aT[h*64:(h+1)*64, :] = pv[h][:64] / pv[h][64]
        # (sum rows moved to partition 0 by DMA: reciprocal /
        #  partition_broadcast only work from base partition 0)
        pv = pv_tiles
        at = atpool.tile([128, QB], bf16, tag=f"at{s}", name=f"at{s}")
        aT_cur[s] = at
        s_sb = sm1pool.tile([HD + 1, 2, QB], fp32, tag="ssb", name="ssb")
        for h in range(2):
            nc.vector.tensor_copy(s_sb[HD : HD + 1, h, :], pv[h][HD : HD + 1, :])
        s0 = sm1pool.tile([1, 2, QB], fp32, tag="s0", name="s0")
        nc.sync.dma_start(s0[:], s_sb[HD : HD + 1, :, :])
        nc.vector.reciprocal(s0[:], s0[:])
        rb = smpool.tile([HD, 2, QB], fp32, tag="rb", name="rb")
        nc.gpsimd.partition_broadcast(rb[:], s0[0:1, :, :])
        nc.vector.tensor_mul(at[0:HD, :], pv[0][0:HD, :], rb[:, 0, :])
        tmp = smpool.tile([HD, QB], bf16, tag="tmp", name="tmp")
        nc.vector.tensor_mul(tmp[:], pv[1][0:HD, :], rb[:, 1, :])
        nc.sync.dma_start(at[HD:128, :], tmp[:])

    def emit_wo(qt, nh, ats):
        # out2 rows for q tile qt (fp16 SBUF staging, DVE/ACT evacuation)
        nsl = slice(nh * 512, (nh + 1) * 512)
        qoff = (qt * 128) % QB
        ps = psmm.tile([128, QB], fp32, tag="mm", name="pso")
        for s in range(NSLOT):
            nc.tensor.matmul(
                ps[:],
                ats[s][:, qoff : qoff + 128],
                wo_sb[:, s, nsl],
                start=(s == 0),
                stop=(s == NSLOT - 1),
            )
        ob = opool.tile([128, QB], fp16, tag="ob", name="ob")
        nc.vector.tensor_copy(ob[:], ps[:])
        nc.sync.dma_start(out2.ap()[qt * 128 : (qt + 1) * 128, nsl], ob[:])

    # ---- pipelined emission ----
    # FIFO of (push_step, kind, fn).  Drained between score k-tiles; FIFO
    # order + forced drains keep every PE instruction's PE-side producers
    # ahead of it in the queue (in-order PE would deadlock otherwise).
    fifo = deque()
    steps = [(qb, s) for qb in range(NQ) for s in range(NSLOT)]

    def drain(n=None):
        while fifo and (n is None or n > 0):
            _, _, fn = fifo.popleft()
            fn()
            if n is not None:
                n -= 1

    def drain_older_than(idx, lag):
        while fifo and fifo[0][0] <= idx - lag:
            _, _, fn = fifo.popleft()
            fn()

    def drain_until_kind_done(kind):
        """Emit FIFO entries (in order) until none of `kind` remain."""
        while any(e[1] == kind for e in fifo):
            _, _, fn = fifo.popleft()
            fn()

    for idx, (qb, s) in enumerate(steps):
        T = Ts[s]
        pos = idx % NSLOT
        # hard ordering: qts chunk qb and pv(qb-1, s) must already be emitted
        # before this step's score matmuls / exp enter the queues
        if pos == 0 and qb > 0:
            drain_until_kind_done(f"qp{qb}")
        drain_until_kind_done(f"pv{qb - 1}_{s}")
        drain_older_than(idx, 3)

        p = ppool.tile([128, T, 2, QB], bf16, tag=f"p{s}", name=f"p{s}")
        for kt in range(T):
            emit_scores_kt(qb, s, kt, p)
            if kt % 2 == 1:
                drain(2 if len(fifo) > 6 else 1)

        # queue follow-on work for this step
        if idx == 0:
            # all of V-proj ahead of the first PV (vsort[0] has T == TMAX)
            for kt in range(TMAX):
                fifo.append((idx, "vp", lambda kt=kt: emit_vproj_kt(kt)))
        pk = f"pv{qb}_{s}"
        fifo.append((idx, pk, lambda a=qb, b=s, pt=p: emit_pv_h(a, b, 0, pt)))
        fifo.append((idx, pk, lambda a=qb, b=s, pt=p: emit_pv_h(a, b, 1, pt)))
        fifo.append((idx, "nm", lambda a=qb, b=s: emit_norm(a, b)))
        if pos == 1 and qb + 1 < NQ:
            fifo.append((idx, f"qp{qb + 1}", lambda c=qb + 1: emit_qproj(c)))
        if pos == 2 and qb + 2 < NQ:
            fifo.append((idx, "ld", lambda c=qb + 2: load_xq(c)))
        if pos == NSLOT - 1:
            for qt in range(qb * (QB // 128), (qb + 1) * (QB // 128)):
                for nh in range(2):
                    fifo.append(
                        (idx, "wo", lambda a=qt, b=nh: emit_wo(a, b, aT_cur))
                    )

    drain()


def build_in_maps(query, key, value, valid_length, Wq, Wk, Wv, Wo):
    """Host-side sharding. Returns (Ts, in_maps)."""
    valid = np.asarray(valid_length).astype(np.int64)
    Ts = tuple(int(-(-v // KT)) for v in valid)
    CKMAX = max(Ts) * KT
    vsort = vsort_order(Ts)

    bf = ml_dtypes.bfloat16
    query = np.asarray(query)
    key = np.asarray(key)
    value = np.asarray(value)
    qTs = [np.ascontiguousarray(query[b].T).astype(bf) for b in range(B)]
    kTs = [np.ascontiguousarray(key[b].T[:, :CKMAX]).astype(bf) for b in range(B)]
    vTs = [np.ascontiguousarray(value[b].T[:, :CKMAX]).astype(bf) for b in range(B)]

    bias = np.zeros((KT, NSLOT), np.float32)
    for s in range(NSLOT):
        rem = int(valid[s]) - (Ts[s] - 1) * KT  # 1..128 valid rows in last tile
        bias[rem:, s] = MASK_BIAS

    Wqb = np.asarray(Wq).astype(bf)
    Wkb = np.asarray(Wk).astype(bf)
    Wvb = np.asarray(Wv).astype(bf)
    Wob = np.asarray(Wo).astype(bf)

    in_maps = []
    for c in range(NCORES):
        beta = c % 4
        heads = core_heads(c)
        hcols = np.concatenate(
            [np.arange(h * HD, (h + 1) * HD) for h in heads]
        )
        # wv columns in vsort slot order (merged V-proj reads a prefix)
        vcols = np.concatenate(
            [
                np.concatenate(
                    [
                        np.arange(heads[2 * s] * HD, (heads[2 * s] + 1) * HD),
                        np.arange(
                            heads[2 * s + 1] * HD, (heads[2 * s + 1] + 1) * HD
                        ),
                    ]
                )
                for s in vsort
            ]
        )
        in_maps.append(
            {
                "qT": qTs[beta],
                "kT": kTs[beta],
                "vT": vTs[beta],
                "wq": np.ascontiguousarray(Wqb[:, hcols]),
                "wk": np.ascontiguousarray(Wkb[:, hcols]),
                "wv": np.ascontiguousarray(Wvb[:, vcols]),
                "wo": np.ascontiguousarray(Wob[hcols, :]),
                "bias": bias,
            }
        )
    return Ts, in_maps


def kernel(query, key, value, valid_length, Wq, Wk, Wv, Wo):
    from concourse.bass_utils import run_bass_kernel_spmd

    Ts, in_maps = build_in_maps(
        query, key, value, valid_length, Wq, Wk, Wv, Wo
    )
    if Ts not in _compiled:
        _compiled[Ts] = _build(Ts)
    nc = _compiled[Ts]

    res = run_bass_kernel_spmd(nc, in_maps, list(range(NCORES)))
    out = np.zeros((B, S, D), np.float32)
    for c in range(NCORES):
        out[c % 4] += res.results[c]["out2"].astype(np.float32)
    return out
